# revision 11
# baseline (speedup 1.0000x reference)
"""Trainium2 Bass kernel for CensorNet (GRU + per-step binary-NLL decoder).

Model (see reference): xp = x @ W_ih^T + b_ih precomputed per step;
recurrence over t = 0..T-2:
    hp = h @ W_hh^T + b_hh
    r = sigmoid(xp_r + hp_r); z = sigmoid(xp_z + hp_z)
    n = tanh(xp_n + r * hp_n)
    h' = (1-z)*n + z*h
    C = sigmoid(h' @ W_dec^T + b_dec)
    nll += -sum(gt[t+1]*log(C+eps) + (1-gt[t+1])*log(1-C+eps))
output = nll / (T*B)

Strategy: shard B=512 across 8 cores (64 each). On each core keep h
transposed ([H=128 partitions, 64 batch]) so the recurrence runs on
[128, 64] tiles with H on partitions. x is DMA'd in natural layout,
transposed on the PE, and the input-side projections are fused into the
gate PSUM accumulation (xp matmul start=True, recurrent matmul
start=False). The per-step decoder logit is one tiny matmul (h^T
stationary, W_dec^T moving) written into a persistent PSUM bank; the
whole NLL is evaluated in one batched pass at the end.
"""

import numpy as np
from contextlib import ExitStack

import concourse.bacc as bacc
import concourse.bass as bass
import concourse.mybir as mybir
import concourse.tile as tile
from concourse.bass_utils import run_bass_kernel_spmd
from concourse.masks import make_identity

T, B, I, H = 512, 512, 128, 128
EPS = 1e-4
NCORES = 8
BL = B // NCORES          # 64 batch per core
NSTEP = T - 1             # 511 recurrence steps
W = 32                    # xp ring window, in steps
CHUNK = 8                 # steps produced per xp chunk
BATCH = 32                # steps per x DMA batch
NCHUNK = T // CHUNK       # 64

f32 = mybir.dt.float32
AF = mybir.ActivationFunctionType
ALU = mybir.AluOpType
AX = mybir.AxisListType

LAST_RESULTS = None       # test harness peeks at this for exec_time_ns


def _body(ctx, tc, aps):
    nc = tc.nc
    x_d, gt_d, wih_d, whh_d, bih_d, bhh_d, wdec_d, bdec_d, out_d = aps

    consts = ctx.enter_context(tc.tile_pool(name="consts", bufs=1))
    xstage = ctx.enter_context(tc.tile_pool(name="xstage", bufs=2))
    xtc = ctx.enter_context(tc.tile_pool(name="xtc", bufs=2))
    rings = ctx.enter_context(tc.tile_pool(name="rings", bufs=1))
    hpool = ctx.enter_context(tc.tile_pool(name="hpool", bufs=3))
    work = ctx.enter_context(tc.tile_pool(name="work", bufs=2))
    final = ctx.enter_context(tc.tile_pool(name="final", bufs=1))
    pg = ctx.enter_context(tc.tile_pool(name="pg", bufs=2, space="PSUM"))
    pgn = ctx.enter_context(tc.tile_pool(name="pgn", bufs=2, space="PSUM"))
    pxt = ctx.enter_context(tc.tile_pool(name="pxt", bufs=1, space="PSUM"))
    pxp = ctx.enter_context(tc.tile_pool(name="pxp", bufs=1, space="PSUM"))
    plog = ctx.enter_context(tc.tile_pool(name="plog", bufs=1, space="PSUM"))

    # ---- constants / weights ----
    wih_sb = consts.tile([I, 3 * H], f32)
    nc.sync.dma_start(wih_sb, wih_d)
    whh_sb = consts.tile([H, 3 * H], f32)
    nc.sync.dma_start(whh_sb, whh_d)
    bih_sb = consts.tile([H, 3], f32)
    nc.sync.dma_start(bih_sb, bih_d)
    bhh_sb = consts.tile([H, 3], f32)
    nc.sync.dma_start(bhh_sb, bhh_d)
    wdec_sb = consts.tile([H, 1], f32)
    nc.sync.dma_start(wdec_sb, wdec_d)
    gt_sb = consts.tile([BL, T], f32)
    nc.sync.dma_start(gt_sb, gt_d)
    bdec_sb = consts.tile([BL, 1], f32)
    nc.gpsimd.dma_start(bdec_sb, bass.AP(bdec_d.tensor, 0, [[0, BL], [1, 1]]))
    ident = consts.tile([128, 128], f32)
    make_identity(nc, ident)

    bias_rz = consts.tile([H, 2], f32)   # b_ih + b_hh for r and z gates
    nc.vector.tensor_add(bias_rz, bih_sb[:, 0:2], bhh_sb[:, 0:2])
    nbdec_sb = consts.tile([BL, 1], f32)
    nc.vector.tensor_scalar_mul(nbdec_sb, bdec_sb, -1.0)
    eps_sb = consts.tile([BL, 1], f32)
    nc.vector.memset(eps_sb, EPS)
    zero_sb = consts.tile([H, 1], f32)
    nc.vector.memset(zero_sb, 0.0)

    # ---- persistent buffers ----
    # xp_rz ring: sigmoid pre-activations' input-side part, biases folded,
    # laid out per step as [r cols | z cols] so one identity-matmul can add
    # it into the gate PSUM and one ACT op computes both sigmoids.
    rz_ring = rings.tile([128, W * 2 * BL], f32)
    xn_ring = rings.tile([128, W * BL], f32)   # W_ih_n @ x_t (b_ih_n in tanh)
    plog_t = plog.tile([BL, 512], f32)         # decoder logits, col = t-1

    x_r = x_d.rearrange("(n p) i -> n p i", p=128)  # n = row-tile index
    stage_ref = [None]

    def produce_chunk(c):
        # fills rz_ring / xn_ring for steps [c*CHUNK, (c+1)*CHUNK)
        if (c * CHUNK) % BATCH == 0:
            st = xstage.tile([128, BATCH * BL // 128, 128], f32, tag="st")
            n0 = (c * CHUNK * BL) // 128
            nc.sync.dma_start(
                st, x_r[n0:n0 + BATCH * BL // 128, :, :].rearrange("n p i -> p n i")
            )
            stage_ref[0] = st
        st = stage_ref[0]
        j0 = (((c * CHUNK) % BATCH) * BL) // 128   # tile offset within stage
        s0 = (c * CHUNK) % W                        # ring slot of first step
        xt = xtc.tile([128, CHUNK * BL], f32, tag="xt")
        for j in range(CHUNK * BL // 128):          # 4 tiles, 2 steps each
            ps = pxt.tile([128, 128], f32, tag="ps")
            nc.tensor.transpose(ps, st[:, j0 + j, :], ident)
            nc.vector.tensor_copy(xt[:, j * 128:(j + 1) * 128], ps)
        for g in (0, 1):                            # r and z input projections
            px = pxp.tile([128, CHUNK * BL], f32, tag="px")
            nc.tensor.matmul(px, wih_sb[:, g * H:(g + 1) * H], xt,
                             start=True, stop=True)
            # ring layout per step s: [r | z] -> 2*BL-wide slots
            dst = rz_ring[:, s0 * 2 * BL:(s0 + CHUNK) * 2 * BL].rearrange(
                "p (s c) -> p s c", c=2 * BL)[:, :, g * BL:(g + 1) * BL]
            nc.vector.tensor_scalar_add(dst, px, bias_rz[:, g:g + 1])
        px = pxp.tile([128, CHUNK * BL], f32, tag="px")
        nc.tensor.matmul(px, wih_sb[:, 2 * H:3 * H], xt, start=True, stop=True)
        nc.vector.tensor_copy(xn_ring[:, s0 * BL:(s0 + CHUNK) * BL], px)

    # ---- prologue ----
    produce_chunk(0)
    produce_chunk(1)

    h_cur = hpool.tile([H, BL], f32, tag="h")
    nc.vector.memset(h_cur, 0.0)

    # ---- recurrence ----
    for t in range(NSTEP):
        if t % CHUNK == 0 and t // CHUNK + 2 < NCHUNK:
            produce_chunk(t // CHUNK + 2)

        s = t % W
        pg_t = pg.tile([128, 2 * BL], f32, tag="pgate")    # r|z bank
        pn_t = pgn.tile([128, BL], f32, tag="pgn")         # n bank (separate
        # so DVE's read of hn doesn't serialize against ACT's sigmoid reads)
        # input-side r|z projections + biases, via identity matmul
        # (independent of h -> scheduled early by the tile scheduler)
        nc.tensor.matmul(pg_t, ident,
                         rz_ring[:, s * 2 * BL:(s + 1) * 2 * BL],
                         start=True, stop=False)
        # recurrent projections accumulate on top
        nc.tensor.matmul(pg_t[:, 0:BL], whh_sb[:, 0:H], h_cur,
                         start=False, stop=False)
        nc.tensor.matmul(pg_t[:, BL:2 * BL], whh_sb[:, H:2 * H], h_cur,
                         start=False, stop=True)
        nc.tensor.matmul(pn_t, whh_sb[:, 2 * H:3 * H], h_cur,
                         start=True, stop=True)
        # decoder logit of h_t pairs with gt[t] -> column t-1
        if t >= 1:
            nc.tensor.matmul(plog_t[:, t - 1:t], h_cur, wdec_sb,
                             start=True, stop=True)

        rz = work.tile([128, 2 * BL], f32, tag="rz")
        nc.scalar.activation(rz, pg_t, AF.Sigmoid, bias=zero_sb)
        zb = work.tile([128, BL], f32, tag="zb")   # 1-z = sigmoid(-s_z)
        nc.scalar.activation(zb, pg_t[:, BL:2 * BL], AF.Sigmoid,
                             bias=zero_sb, scale=-1.0)

        u = work.tile([128, BL], f32, tag="u")     # (hn + b_hh_n) * r
        nc.vector.scalar_tensor_tensor(
            u, pn_t, bhh_sb[:, 2:3], rz[:, 0:BL],
            op0=ALU.add, op1=ALU.mult,
        )
        v = work.tile([128, BL], f32, tag="v")
        nc.vector.tensor_add(v, u, xn_ring[:, s * BL:(s + 1) * BL])
        t1 = work.tile([128, BL], f32, tag="t1")   # z * h (off critical path)
        nc.vector.tensor_mul(t1, rz[:, BL:2 * BL], h_cur)

        n_t = work.tile([128, BL], f32, tag="nt")  # tanh(v + b_ih_n)
        nc.scalar.activation(n_t, v, AF.Tanh, bias=bih_sb[:, 2:3])

        t2 = work.tile([128, BL], f32, tag="t2")   # (1-z) * n
        nc.vector.tensor_mul(t2, zb, n_t)
        h_next = hpool.tile([H, BL], f32, tag="h")
        nc.vector.tensor_add(h_next, t2, t1)
        h_cur = h_next

    # logit of the final hidden state
    nc.tensor.matmul(plog_t[:, NSTEP - 1:NSTEP], h_cur, wdec_sb,
                     start=True, stop=True)

    # ---- batched NLL ----
    c1 = final.tile([BL, NSTEP], f32)
    nc.scalar.activation(c1, plog_t[:, 0:NSTEP], AF.Sigmoid, bias=bdec_sb)
    c2 = final.tile([BL, NSTEP], f32)   # 1 - C = sigmoid(-logit)
    nc.scalar.activation(c2, plog_t[:, 0:NSTEP], AF.Sigmoid,
                         bias=nbdec_sb, scale=-1.0)
    l1 = final.tile([BL, NSTEP], f32)
    nc.scalar.activation(l1, c1, AF.Ln, bias=eps_sb)
    l2 = final.tile([BL, NSTEP], f32)
    nc.scalar.activation(l2, c2, AF.Ln, bias=eps_sb)
    m = final.tile([BL, NSTEP], f32)
    nc.vector.tensor_sub(m, l1, l2)
    m2 = final.tile([BL, NSTEP], f32)
    nc.vector.tensor_mul(m2, m, gt_sb[:, 1:T])
    s_t = final.tile([BL, NSTEP], f32)
    nc.vector.tensor_add(s_t, m2, l2)
    red = final.tile([BL, 1], f32)
    nc.vector.tensor_reduce(red, s_t, axis=AX.X, op=ALU.add)
    nred = final.tile([BL, 1], f32)
    nc.vector.tensor_scalar_mul(nred, red, -1.0)
    nc.sync.dma_start(out_d, nred)


_BUILT = None


def _build():
    global _BUILT
    if _BUILT is not None:
        return _BUILT
    nc = bacc.Bacc("TRN2", target_bir_lowering=False, debug=False,
                   enable_asserts=False, num_devices=NCORES)
    aps = (
        nc.dram_tensor("x", [T * BL, I], f32, kind="ExternalInput").ap(),
        nc.dram_tensor("gt_t", [BL, T], f32, kind="ExternalInput").ap(),
        nc.dram_tensor("w_ih_t", [I, 3 * H], f32, kind="ExternalInput").ap(),
        nc.dram_tensor("w_hh_t", [H, 3 * H], f32, kind="ExternalInput").ap(),
        nc.dram_tensor("b_ih_t", [H, 3], f32, kind="ExternalInput").ap(),
        nc.dram_tensor("b_hh_t", [H, 3], f32, kind="ExternalInput").ap(),
        nc.dram_tensor("w_dec_t", [H, 1], f32, kind="ExternalInput").ap(),
        nc.dram_tensor("b_dec", [1, 1], f32, kind="ExternalInput").ap(),
        nc.dram_tensor("nll_part", [BL, 1], f32, kind="ExternalOutput").ap(),
    )
    with tile.TileContext(nc) as tc, ExitStack() as ctx:
        _body(ctx, tc, aps)
    nc.compile()
    _BUILT = nc
    return nc


def kernel(x, gt, W_ih, W_hh, b_ih, b_hh, W_dec, b_dec):
    global LAST_RESULTS
    x = np.asarray(x, dtype=np.float32)
    gt = np.asarray(gt, dtype=np.float32)
    W_ih = np.asarray(W_ih, dtype=np.float32)
    W_hh = np.asarray(W_hh, dtype=np.float32)
    b_ih = np.asarray(b_ih, dtype=np.float32)
    b_hh = np.asarray(b_hh, dtype=np.float32)
    W_dec = np.asarray(W_dec, dtype=np.float32)
    b_dec = np.asarray(b_dec, dtype=np.float32)

    nc = _build()

    shared = {
        "w_ih_t": np.ascontiguousarray(W_ih.T),
        "w_hh_t": np.ascontiguousarray(W_hh.T),
        "b_ih_t": np.ascontiguousarray(b_ih.reshape(3, H).T),
        "b_hh_t": np.ascontiguousarray(b_hh.reshape(3, H).T),
        "w_dec_t": np.ascontiguousarray(W_dec.reshape(1, H).T),
        "b_dec": np.ascontiguousarray(b_dec.reshape(1, 1)),
    }
    in_maps = []
    for c in range(NCORES):
        b0 = c * BL
        in_maps.append(dict(
            shared,
            x=np.ascontiguousarray(x[:, b0:b0 + BL, :]).reshape(T * BL, I),
            gt_t=np.ascontiguousarray(gt[:, b0:b0 + BL, 0].T),
        ))

    res = run_bass_kernel_spmd(nc, in_maps, core_ids=list(range(NCORES)))
    LAST_RESULTS = res
    total = sum(float(r["nll_part"].sum(dtype=np.float64)) for r in res.results)
    return np.float32(total / float(T * B))


# revision 23
# speedup vs baseline: 33.6040x; 33.6040x over previous
"""Trainium2 Bass kernel for CensorNet (GRU + per-step binary-NLL decoder).

Model (see reference): xp = x @ W_ih^T + b_ih precomputed per step;
recurrence over t = 0..T-2:
    hp = h @ W_hh^T + b_hh
    r = sigmoid(xp_r + hp_r); z = sigmoid(xp_z + hp_z)
    n = tanh(xp_n + r * hp_n)
    h' = (1-z)*n + z*h
    C = sigmoid(h' @ W_dec^T + b_dec)
    nll += -sum(gt[t+1]*log(C+eps) + (1-gt[t+1])*log(1-C+eps))
output = nll / (T*B)

Strategy: shard B=512 across 8 cores (64 each), data parallel; weights
replicated; host only does layout transforms and the final partial-sum
gather. Per core, h lives transposed ([H=128 partitions, 64 batch]) so
the recurrence runs on [128, 64] tiles with H on partitions:

- x is host-cast to bf16 and loaded pre-transposed via the DMA xbar
  transpose; the input-side projections for all three gates are computed
  chunk-wise (8 steps at a time) by bf16 matmuls into rings, with gate
  biases folded in during PSUM evacuation.
- Each step, one identity matmul adds the r|z input projections into the
  gate PSUM bank, the recurrent matmuls accumulate on top, and a single
  sigmoid ACTIVATE covers both r and z. The n-gate PSUM lives in its own
  bank so the vector engine's reads don't serialize against the scalar
  engine's sigmoid reads (same-bank PSUM accesses serialize).
- The per-step decoder logit is one tiny matmul (h^T stationary, W_dec^T
  moving) into a persistent PSUM bank; the whole NLL is evaluated in one
  batched pass at the end (sigmoid/log on [64, 511] tiles).
"""

import numpy as np
import ml_dtypes
from contextlib import ExitStack

import concourse.bacc as bacc
import concourse.bass as bass
import concourse.mybir as mybir
import concourse.tile as tile
from concourse.bass_utils import run_bass_kernel_spmd
from concourse.masks import make_identity

T, B, I, H = 512, 512, 128, 128
EPS = 1e-4
NCORES = 8
BL = B // NCORES          # 64 batch per core
NSTEP = T - 1             # 511 recurrence steps
W = 32                    # xp ring window, in steps
CHUNK = 8                 # steps produced per xp chunk
NCHUNK = T // CHUNK       # 64

f32 = mybir.dt.float32
bf16 = mybir.dt.bfloat16
AF = mybir.ActivationFunctionType
ALU = mybir.AluOpType
AX = mybir.AxisListType

LAST_RESULTS = None       # test harness peeks at this for exec_time_ns


def _body(ctx, tc, aps, reps=1):
    nc = tc.nc
    x_d, gt_d, wih_d, whh_d, bih_d, bhh_d, wdec_d, bdec_d, out_d = aps

    consts = ctx.enter_context(tc.tile_pool(name="consts", bufs=1))
    xtc = ctx.enter_context(tc.tile_pool(name="xtc", bufs=2))
    rings = ctx.enter_context(tc.tile_pool(name="rings", bufs=1))
    hpool = ctx.enter_context(tc.tile_pool(name="hpool", bufs=3))
    work = ctx.enter_context(tc.tile_pool(name="work", bufs=2))
    final = ctx.enter_context(tc.tile_pool(name="final", bufs=1))
    pg = ctx.enter_context(tc.tile_pool(name="pg", bufs=2, space="PSUM"))
    pgn = ctx.enter_context(tc.tile_pool(name="pgn", bufs=2, space="PSUM"))
    pxp = ctx.enter_context(tc.tile_pool(name="pxp", bufs=2, space="PSUM"))
    plog = ctx.enter_context(tc.tile_pool(name="plog", bufs=1, space="PSUM"))

    # ---- constants / weights ----
    wih_sb = consts.tile([I, 3 * H], bf16)
    nc.sync.dma_start(wih_sb, wih_d)
    whh_sb = consts.tile([H, 3 * H], bf16)
    nc.sync.dma_start(whh_sb, whh_d)
    bih_sb = consts.tile([H, 3], f32)
    nc.sync.dma_start(bih_sb, bih_d)
    bhh_sb = consts.tile([H, 3], f32)
    nc.sync.dma_start(bhh_sb, bhh_d)
    wdec_sb = consts.tile([H, 1], bf16)
    nc.sync.dma_start(wdec_sb, wdec_d)
    gt_sb = consts.tile([BL, T], f32)
    nc.sync.dma_start(gt_sb, gt_d)
    bdec_sb = consts.tile([BL, 1], f32)
    nc.gpsimd.dma_start(bdec_sb, bass.AP(bdec_d.tensor, 0, [[0, BL], [1, 1]]))
    ident = consts.tile([128, 128], bf16)
    make_identity(nc, ident)

    bias_rz = consts.tile([H, 2], f32)   # b_ih + b_hh for r and z gates
    nc.vector.tensor_add(bias_rz, bih_sb[:, 0:2], bhh_sb[:, 0:2])
    nbdec_sb = consts.tile([BL, 1], f32)
    nc.vector.tensor_scalar_mul(nbdec_sb, bdec_sb, -1.0)
    eps_sb = consts.tile([BL, 1], f32)
    nc.vector.memset(eps_sb, EPS)
    zero_sb = consts.tile([H, 1], f32)
    nc.vector.memset(zero_sb, 0.0)
    # b_hh_n broadcast along the batch dim, added into the n-gate PSUM by
    # an identity matmul so the u-multiply is a plain tensor_tensor
    bcast_bhhn = consts.tile([H, BL], bf16)
    nc.vector.memset(bcast_bhhn, 0.0)
    nc.vector.tensor_scalar_add(bcast_bhhn, bcast_bhhn, bhh_sb[:, 2:3])

    # ---- persistent buffers ----
    # xp_rz ring: input-side sigmoid pre-activations, biases folded, laid
    # out per step as [r cols | z cols] so one identity-matmul adds them
    # into the gate PSUM and one ACT op computes both sigmoids.
    rz_ring = rings.tile([128, W * 2 * BL], bf16)
    xn_ring = rings.tile([128, W * BL], bf16)  # W_ih_n @ x_t (b_ih_n in tanh)
    plog_t = plog.tile([BL, 512], f32)         # decoder logits, col = t-1

    def produce_chunk(c, part):
        # fills rz_ring / xn_ring for steps [c*CHUNK, (c+1)*CHUNK); split in
        # two 4-step parts (part=0/1) whose matmul/evac pieces are small
        # enough for the scheduler to slot into chain gaps
        s0 = (c * CHUNK) % W
        r0 = c * CHUNK * BL
        hc = CHUNK * BL // 2                    # elements per part (256)
        if part == 0:
            xt = xtc.tile([128, CHUNK * BL], bf16, tag="xt")
            xt_parts[0] = xt
            nc.sync.dma_start_transpose(xt, x_d[r0:r0 + CHUNK * BL, :])
        xt = xt_parts[0]
        sl = slice(part * hc, (part + 1) * hc)
        s0p = s0 + part * (CHUNK // 2)
        for g in (0, 1):                        # r and z input projections
            px = pxp.tile([128, hc], f32, tag="px")
            nc.tensor.matmul(px, wih_sb[:, g * H:(g + 1) * H], xt[:, sl],
                             start=True, stop=True)
            # ring layout per step s: [r | z] -> 2*BL-wide slots
            dst = rz_ring[:, s0p * 2 * BL:(s0p + CHUNK // 2) * 2 * BL].rearrange(
                "p (s c) -> p s c", c=2 * BL)[:, :, g * BL:(g + 1) * BL]
            nc.vector.tensor_scalar_add(dst, px, bias_rz[:, g:g + 1])
        px = pxp.tile([128, hc], f32, tag="px")
        nc.tensor.matmul(px, wih_sb[:, 2 * H:3 * H], xt[:, sl],
                         start=True, stop=True)
        nc.vector.tensor_copy(xn_ring[:, s0p * BL:(s0p + CHUNK // 2) * BL], px)

    xt_parts = [None]

    def compute():
        # ---- prologue ----
        for c in (0, 1):
            produce_chunk(c, 0)
            produce_chunk(c, 1)

        # h_t is carried as the unmaterialized pair (a_t, b_t) with
        # h_t = a_t + b_t, a = z*h_prev (ready early in the producing
        # step), b = (1-z)*n (ready late). The gate matmuls run separately
        # on a and b, so only the b-matmuls sit on the critical path; the
        # materialized h (needed for z*h and the decoder) is formed
        # off-chain while the sigmoid runs.
        a_cur = None       # step 0: h_0 = 0, no recurrent contribution
        b_cur = None

        # ---- recurrence ----
        for t in range(NSTEP):
            s = t % W
            pg_t = pg.tile([128, 2 * BL], f32, tag="pgate")  # r|z bank
            pn_t = pgn.tile([128, BL], f32, tag="pgn")       # n bank
            # input-side projections + biases via identity matmuls
            # (independent of h -> scheduled early, off the critical path)
            only = t == 0
            nc.tensor.matmul(pg_t, ident,
                             rz_ring[:, s * 2 * BL:(s + 1) * 2 * BL],
                             start=True, stop=only)
            nc.tensor.matmul(pn_t, ident, bcast_bhhn, start=True, stop=only)
            if t >= 1:
                # a-side recurrent matmuls (a ready since mid-prev-step)
                nc.tensor.matmul(pg_t[:, 0:BL], whh_sb[:, 0:H], a_cur,
                                 start=False, stop=False)
                nc.tensor.matmul(pg_t[:, BL:2 * BL], whh_sb[:, H:2 * H],
                                 a_cur, start=False, stop=False)
                nc.tensor.matmul(pn_t, whh_sb[:, 2 * H:3 * H], a_cur,
                                 start=False, stop=False)
                # b-side recurrent matmuls -- the critical-path segment
                nc.tensor.matmul(pg_t[:, 0:BL], whh_sb[:, 0:H], b_cur,
                                 start=False, stop=False)
                nc.tensor.matmul(pg_t[:, BL:2 * BL], whh_sb[:, H:2 * H],
                                 b_cur, start=False, stop=True)
                nc.tensor.matmul(pn_t, whh_sb[:, 2 * H:3 * H], b_cur,
                                 start=False, stop=True)
                h_mat = hpool.tile([H, BL], bf16, tag="h")   # h_t = a+b
                nc.vector.tensor_add(h_mat, a_cur, b_cur)
                # decoder logit of h_t pairs with gt[t] -> column t-1
                nc.tensor.matmul(plog_t[:, t - 1:t], h_mat, wdec_sb,
                                 start=True, stop=True)

            rz = work.tile([128, 2 * BL], bf16, tag="rz")
            nc.scalar.activation(rz, pg_t, AF.Sigmoid, bias=zero_sb)

            u = work.tile([128, BL], bf16, tag="u")    # (hn + b_hh_n) * r
            nc.vector.tensor_mul(u, pn_t, rz[:, 0:BL])
            v = work.tile([128, BL], bf16, tag="v")
            nc.vector.tensor_add(v, u, xn_ring[:, s * BL:(s + 1) * BL])
            n_t = work.tile([128, BL], bf16, tag="nt")  # tanh(v + b_ih_n)
            nc.scalar.activation(n_t, v, AF.Tanh, bias=bih_sb[:, 2:3])

            # off-critical-path ops; emitted after tanh so the scheduler
            # keeps u->v ahead of them on the vector engine
            zb = work.tile([128, BL], bf16, tag="zb")  # 1-z = (z-1)*(-1)
            nc.vector.tensor_scalar(zb, rz[:, BL:2 * BL], 1.0, -1.0,
                                    op0=ALU.subtract, op1=ALU.mult)
            a_next = hpool.tile([H, BL], bf16, tag="a")  # z * h_t
            if t >= 1:
                nc.vector.tensor_mul(a_next, rz[:, BL:2 * BL], h_mat)
            else:
                nc.vector.memset(a_next, 0.0)

            b_next = hpool.tile([H, BL], bf16, tag="b")  # (1-z) * n
            nc.vector.tensor_mul(b_next, zb, n_t)
            a_cur, b_cur = a_next, b_next

            # produce the xp chunk two windows ahead, half a chunk at a
            # time; emitted after the step so its matmuls queue behind
            # this step's gate matmuls on the PE
            if t % CHUNK == 0 and t // CHUNK + 2 < NCHUNK:
                produce_chunk(t // CHUNK + 2, 0)
            elif t % CHUNK == CHUNK // 2 and t // CHUNK + 2 < NCHUNK:
                produce_chunk(t // CHUNK + 2, 1)

        # logit of the final hidden state h_{T-1} = a + b
        h_last = hpool.tile([H, BL], bf16, tag="h")
        nc.vector.tensor_add(h_last, a_cur, b_cur)
        nc.tensor.matmul(plog_t[:, NSTEP - 1:NSTEP], h_last, wdec_sb,
                         start=True, stop=True)

        # ---- batched NLL ----
        c1 = final.tile([BL, NSTEP], f32)
        nc.scalar.activation(c1, plog_t[:, 0:NSTEP], AF.Sigmoid, bias=bdec_sb)
        c2 = final.tile([BL, NSTEP], f32)   # 1 - C = sigmoid(-logit)
        nc.scalar.activation(c2, plog_t[:, 0:NSTEP], AF.Sigmoid,
                             bias=nbdec_sb, scale=-1.0)
        l1 = final.tile([BL, NSTEP], f32)
        nc.scalar.activation(l1, c1, AF.Ln, bias=eps_sb)
        l2 = final.tile([BL, NSTEP], f32)
        nc.scalar.activation(l2, c2, AF.Ln, bias=eps_sb)
        m = final.tile([BL, NSTEP], f32)
        nc.vector.tensor_sub(m, l1, l2)
        m2 = final.tile([BL, NSTEP], f32)
        nc.vector.tensor_mul(m2, m, gt_sb[:, 1:T])
        s_t = final.tile([BL, NSTEP], f32)
        nc.vector.tensor_add(s_t, m2, l2)
        red = final.tile([BL, 1], f32)
        nc.vector.tensor_reduce(red, s_t, axis=AX.X, op=ALU.add)
        nred = final.tile([BL, 1], f32)
        nc.vector.tensor_scalar_mul(nred, red, -1.0)
        nc.sync.dma_start(out_d, nred)

    if reps == 1:
        compute()
    else:
        with tc.For_i(0, reps, 1):
            compute()


_BUILT = {}


def _build(reps=1):
    if reps in _BUILT:
        return _BUILT[reps]
    nc = bacc.Bacc("TRN2", target_bir_lowering=False, debug=False,
                   enable_asserts=False, num_devices=NCORES)
    aps = (
        nc.dram_tensor("x", [T * BL, I], bf16, kind="ExternalInput").ap(),
        nc.dram_tensor("gt_t", [BL, T], f32, kind="ExternalInput").ap(),
        nc.dram_tensor("w_ih_t", [I, 3 * H], bf16, kind="ExternalInput").ap(),
        nc.dram_tensor("w_hh_t", [H, 3 * H], bf16, kind="ExternalInput").ap(),
        nc.dram_tensor("b_ih_t", [H, 3], f32, kind="ExternalInput").ap(),
        nc.dram_tensor("b_hh_t", [H, 3], f32, kind="ExternalInput").ap(),
        nc.dram_tensor("w_dec_t", [H, 1], bf16, kind="ExternalInput").ap(),
        nc.dram_tensor("b_dec", [1, 1], f32, kind="ExternalInput").ap(),
        nc.dram_tensor("nll_part", [BL, 1], f32, kind="ExternalOutput").ap(),
    )
    with tile.TileContext(nc) as tc, ExitStack() as ctx:
        _body(ctx, tc, aps, reps=reps)
    nc.compile()
    _BUILT[reps] = nc
    return nc


def kernel(x, gt, W_ih, W_hh, b_ih, b_hh, W_dec, b_dec):
    global LAST_RESULTS
    x = np.asarray(x, dtype=np.float32)
    gt = np.asarray(gt, dtype=np.float32)
    W_ih = np.asarray(W_ih, dtype=np.float32)
    W_hh = np.asarray(W_hh, dtype=np.float32)
    b_ih = np.asarray(b_ih, dtype=np.float32)
    b_hh = np.asarray(b_hh, dtype=np.float32)
    W_dec = np.asarray(W_dec, dtype=np.float32)
    b_dec = np.asarray(b_dec, dtype=np.float32)

    nc = _build()

    bf = ml_dtypes.bfloat16
    shared = {
        "w_ih_t": np.ascontiguousarray(W_ih.T).astype(bf),
        "w_hh_t": np.ascontiguousarray(W_hh.T).astype(bf),
        "b_ih_t": np.ascontiguousarray(b_ih.reshape(3, H).T),
        "b_hh_t": np.ascontiguousarray(b_hh.reshape(3, H).T),
        "w_dec_t": np.ascontiguousarray(W_dec.reshape(1, H).T).astype(bf),
        "b_dec": np.ascontiguousarray(b_dec.reshape(1, 1)),
    }
    in_maps = []
    for c in range(NCORES):
        b0 = c * BL
        in_maps.append(dict(
            shared,
            x=np.ascontiguousarray(x[:, b0:b0 + BL, :]).reshape(
                T * BL, I).astype(bf),
            gt_t=np.ascontiguousarray(gt[:, b0:b0 + BL, 0].T),
        ))

    res = run_bass_kernel_spmd(nc, in_maps, core_ids=list(range(NCORES)))
    LAST_RESULTS = res
    total = sum(float(r["nll_part"].sum(dtype=np.float64)) for r in res.results)
    return np.float32(total / float(T * B))


# revision 25
# speedup vs baseline: 82.6971x; 2.4609x over previous
"""Trainium2 Bass kernel for CensorNet (GRU + per-step binary-NLL decoder).

Model (see reference): xp = x @ W_ih^T + b_ih precomputed per step;
recurrence over t = 0..T-2:
    hp = h @ W_hh^T + b_hh
    r = sigmoid(xp_r + hp_r); z = sigmoid(xp_z + hp_z)
    n = tanh(xp_n + r * hp_n)
    h' = (1-z)*n + z*h
    C = sigmoid(h' @ W_dec^T + b_dec)
    nll += -sum(gt[t+1]*log(C+eps) + (1-gt[t+1])*log(1-C+eps))
output = nll / (T*B)

Strategy: shard B=512 across 8 cores (64 each), data parallel; weights
replicated; host only does layout transforms and the final partial-sum
gather. Per core, h lives transposed ([H=128 partitions, 64 batch]) so
the recurrence runs on [128, 64] tiles with H on partitions:

- x is host-cast to bf16 and loaded pre-transposed via the DMA xbar
  transpose; the input-side projections for all three gates are computed
  chunk-wise (8 steps at a time) by bf16 matmuls into rings, with gate
  biases folded in during PSUM evacuation.
- Each step, one identity matmul adds the r|z input projections into the
  gate PSUM bank, the recurrent matmuls accumulate on top, and a single
  sigmoid ACTIVATE covers both r and z. The n-gate PSUM lives in its own
  bank so the vector engine's reads don't serialize against the scalar
  engine's sigmoid reads (same-bank PSUM accesses serialize).
- The per-step decoder logit is one tiny matmul (h^T stationary, W_dec^T
  moving) into a persistent PSUM bank; the whole NLL is evaluated in one
  batched pass at the end (sigmoid/log on [64, 511] tiles).
"""

import numpy as np
import ml_dtypes
from contextlib import ExitStack

import concourse.bacc as bacc
import concourse.bass as bass
import concourse.mybir as mybir
import concourse.tile as tile
from concourse.bass_utils import run_bass_kernel_spmd
from concourse.masks import make_identity
from concourse.tile_rust import add_dep_helper

T, B, I, H = 512, 512, 128, 128
EPS = 1e-4
NCORES = 8
BL = B // NCORES          # 64 batch per core
NSTEP = T - 1             # 511 recurrence steps
W = 32                    # xp ring window, in steps
CHUNK = 8                 # steps produced per xp chunk
NCHUNK = T // CHUNK       # 64

f32 = mybir.dt.float32
bf16 = mybir.dt.bfloat16
AF = mybir.ActivationFunctionType
ALU = mybir.AluOpType
AX = mybir.AxisListType

LAST_RESULTS = None       # test harness peeks at this for exec_time_ns


def _body(ctx, tc, aps, reps=1):
    nc = tc.nc
    x_d, gt_d, wih_d, whh_d, bih_d, bhh_d, wdec_d, bdec_d, out_d = aps

    consts = ctx.enter_context(tc.tile_pool(name="consts", bufs=1))
    xtc = ctx.enter_context(tc.tile_pool(name="xtc", bufs=2))
    rings = ctx.enter_context(tc.tile_pool(name="rings", bufs=1))
    hpool = ctx.enter_context(tc.tile_pool(name="hpool", bufs=3))
    work = ctx.enter_context(tc.tile_pool(name="work", bufs=2))
    final = ctx.enter_context(tc.tile_pool(name="final", bufs=1))
    pg = ctx.enter_context(tc.tile_pool(name="pg", bufs=2, space="PSUM"))
    pgn = ctx.enter_context(tc.tile_pool(name="pgn", bufs=2, space="PSUM"))
    pxp = ctx.enter_context(tc.tile_pool(name="pxp", bufs=2, space="PSUM"))
    plog = ctx.enter_context(tc.tile_pool(name="plog", bufs=1, space="PSUM"))

    # ---- constants / weights ----
    wih_sb = consts.tile([I, 3 * H], bf16)
    nc.sync.dma_start(wih_sb, wih_d)
    whh_sb = consts.tile([H, 3 * H], bf16)
    nc.sync.dma_start(whh_sb, whh_d)
    bih_sb = consts.tile([H, 3], f32)
    nc.sync.dma_start(bih_sb, bih_d)
    bhh_sb = consts.tile([H, 3], f32)
    nc.sync.dma_start(bhh_sb, bhh_d)
    wdec_sb = consts.tile([H, 1], bf16)
    nc.sync.dma_start(wdec_sb, wdec_d)
    gt_sb = consts.tile([BL, T], f32)
    nc.sync.dma_start(gt_sb, gt_d)
    bdec_sb = consts.tile([BL, 1], f32)
    nc.gpsimd.dma_start(bdec_sb, bass.AP(bdec_d.tensor, 0, [[0, BL], [1, 1]]))
    ident = consts.tile([128, 128], bf16)
    make_identity(nc, ident)

    bias_rz = consts.tile([H, 2], f32)   # b_ih + b_hh for r and z gates
    nc.vector.tensor_add(bias_rz, bih_sb[:, 0:2], bhh_sb[:, 0:2])
    nbdec_sb = consts.tile([BL, 1], f32)
    nc.vector.tensor_scalar_mul(nbdec_sb, bdec_sb, -1.0)
    eps_sb = consts.tile([BL, 1], f32)
    nc.vector.memset(eps_sb, EPS)
    zero_sb = consts.tile([H, 1], f32)
    nc.vector.memset(zero_sb, 0.0)
    # b_hh_n broadcast along the batch dim, added into the n-gate PSUM by
    # an identity matmul so the u-multiply is a plain tensor_tensor
    bcast_bhhn = consts.tile([H, BL], bf16)
    nc.vector.memset(bcast_bhhn, 0.0)
    nc.vector.tensor_scalar_add(bcast_bhhn, bcast_bhhn, bhh_sb[:, 2:3])

    # ---- persistent buffers ----
    # xp_rz ring: input-side sigmoid pre-activations, biases folded, laid
    # out per step as [r cols | z cols] so one identity-matmul adds them
    # into the gate PSUM and one ACT op computes both sigmoids.
    rz_ring = rings.tile([128, W * 2 * BL], bf16)
    xn_ring = rings.tile([128, W * BL], bf16)  # W_ih_n @ x_t (b_ih_n in tanh)
    plog_t = plog.tile([BL, 512], f32)         # decoder logits, col = t-1

    def produce_chunk(c, part):
        # fills rz_ring / xn_ring for steps [c*CHUNK, (c+1)*CHUNK); split in
        # two 4-step parts (part=0/1) whose matmul/evac pieces are small
        # enough for the scheduler to slot into chain gaps
        s0 = (c * CHUNK) % W
        r0 = c * CHUNK * BL
        hc = CHUNK * BL // 2                    # elements per part (256)
        if part == 0:
            xt = xtc.tile([128, CHUNK * BL], bf16, tag="xt")
            xt_parts[0] = xt
            nc.sync.dma_start_transpose(xt, x_d[r0:r0 + CHUNK * BL, :])
        xt = xt_parts[0]
        sl = slice(part * hc, (part + 1) * hc)
        s0p = s0 + part * (CHUNK // 2)
        for g in (0, 1):                        # r and z input projections
            px = pxp.tile([128, hc], f32, tag="px")
            nc.tensor.matmul(px, wih_sb[:, g * H:(g + 1) * H], xt[:, sl],
                             start=True, stop=True)
            # ring layout per step s: [r | z] -> 2*BL-wide slots
            dst = rz_ring[:, s0p * 2 * BL:(s0p + CHUNK // 2) * 2 * BL].rearrange(
                "p (s c) -> p s c", c=2 * BL)[:, :, g * BL:(g + 1) * BL]
            nc.vector.tensor_scalar_add(dst, px, bias_rz[:, g:g + 1])
        px = pxp.tile([128, hc], f32, tag="px")
        nc.tensor.matmul(px, wih_sb[:, 2 * H:3 * H], xt[:, sl],
                         start=True, stop=True)
        nc.vector.tensor_copy(xn_ring[:, s0p * BL:(s0p + CHUNK // 2) * BL], px)

    xt_parts = [None]

    def compute():
        # ---- prologue ----
        for c in (0, 1):
            produce_chunk(c, 0)
            produce_chunk(c, 1)

        # h_t is carried as the unmaterialized pair (a_t, b_t) with
        # h_t = a_t + b_t, a = z*h_prev (ready early in the producing
        # step), b = (1-z)*n (ready late). The gate matmuls run separately
        # on a and b, so only the b-matmuls sit on the critical path; the
        # materialized h (needed for z*h and the decoder) is formed
        # off-chain while the sigmoid runs.
        a_cur = None       # step 0: h_0 = 0, no recurrent contribution
        b_cur = None

        # ---- recurrence ----
        for t in range(NSTEP):
            s = t % W
            pg_t = pg.tile([128, 2 * BL], f32, tag="pgate")  # r|z bank
            pn_t = pgn.tile([128, BL], f32, tag="pgn")       # n bank
            # input-side projections + biases via identity matmuls
            # (independent of h -> scheduled early, off the critical path)
            only = t == 0
            nc.tensor.matmul(pg_t, ident,
                             rz_ring[:, s * 2 * BL:(s + 1) * 2 * BL],
                             start=True, stop=only)
            nc.tensor.matmul(pn_t, ident, bcast_bhhn, start=True, stop=only)
            if t >= 1:
                # a-side recurrent matmuls (a ready since mid-prev-step)
                nc.tensor.matmul(pg_t[:, 0:BL], whh_sb[:, 0:H], a_cur,
                                 start=False, stop=False)
                nc.tensor.matmul(pg_t[:, BL:2 * BL], whh_sb[:, H:2 * H],
                                 a_cur, start=False, stop=False)
                nc.tensor.matmul(pn_t, whh_sb[:, 2 * H:3 * H], a_cur,
                                 start=False, stop=False)
                # b-side recurrent matmuls -- the critical-path segment
                nc.tensor.matmul(pg_t[:, 0:BL], whh_sb[:, 0:H], b_cur,
                                 start=False, stop=False)
                nc.tensor.matmul(pg_t[:, BL:2 * BL], whh_sb[:, H:2 * H],
                                 b_cur, start=False, stop=True)
                nc.tensor.matmul(pn_t, whh_sb[:, 2 * H:3 * H], b_cur,
                                 start=False, stop=True)
                h_mat = hpool.tile([H, BL], bf16, tag="h")   # h_t = a+b
                nc.vector.tensor_add(h_mat, a_cur, b_cur)
                # decoder logit of h_t pairs with gt[t] -> column t-1
                nc.tensor.matmul(plog_t[:, t - 1:t], h_mat, wdec_sb,
                                 start=True, stop=True)

            rz = work.tile([128, 2 * BL], bf16, tag="rz")
            nc.scalar.activation(rz, pg_t, AF.Sigmoid, bias=zero_sb)

            u = work.tile([128, BL], bf16, tag="u")    # (hn + b_hh_n) * r
            nc.vector.tensor_mul(u, pn_t, rz[:, 0:BL])
            v = work.tile([128, BL], bf16, tag="v")
            v_inst = nc.vector.tensor_add(v, u, xn_ring[:, s * BL:(s + 1) * BL])
            n_t = work.tile([128, BL], bf16, tag="nt")  # tanh(v + b_ih_n)
            nc.scalar.activation(n_t, v, AF.Tanh, bias=bih_sb[:, 2:3])

            # off-critical-path ops; ordered after v so the scheduler keeps
            # the u->v critical pair at the head of the vector engine queue
            zb = work.tile([128, BL], bf16, tag="zb")  # 1-z = (z-1)*(-1)
            zb_inst = nc.vector.tensor_scalar(zb, rz[:, BL:2 * BL], 1.0, -1.0,
                                              op0=ALU.subtract, op1=ALU.mult)
            add_dep_helper(zb_inst.ins, v_inst.ins, False,
                           "keep zb off the u->v critical path")
            a_next = hpool.tile([H, BL], bf16, tag="a")  # z * h_t
            if t >= 1:
                nc.vector.tensor_mul(a_next, rz[:, BL:2 * BL], h_mat)
            else:
                nc.vector.memset(a_next, 0.0)

            b_next = hpool.tile([H, BL], bf16, tag="b")  # (1-z) * n
            nc.vector.tensor_mul(b_next, zb, n_t)
            a_cur, b_cur = a_next, b_next

            # produce the xp chunk two windows ahead, half a chunk at a
            # time; emitted after the step so its matmuls queue behind
            # this step's gate matmuls on the PE
            if t % CHUNK == 0 and t // CHUNK + 2 < NCHUNK:
                produce_chunk(t // CHUNK + 2, 0)
            elif t % CHUNK == CHUNK // 2 and t // CHUNK + 2 < NCHUNK:
                produce_chunk(t // CHUNK + 2, 1)

        # logit of the final hidden state h_{T-1} = a + b
        h_last = hpool.tile([H, BL], bf16, tag="h")
        nc.vector.tensor_add(h_last, a_cur, b_cur)
        nc.tensor.matmul(plog_t[:, NSTEP - 1:NSTEP], h_last, wdec_sb,
                         start=True, stop=True)

        # ---- batched NLL ----
        c1 = final.tile([BL, NSTEP], f32)
        nc.scalar.activation(c1, plog_t[:, 0:NSTEP], AF.Sigmoid, bias=bdec_sb)
        c2 = final.tile([BL, NSTEP], f32)   # 1 - C = sigmoid(-logit)
        nc.scalar.activation(c2, plog_t[:, 0:NSTEP], AF.Sigmoid,
                             bias=nbdec_sb, scale=-1.0)
        l1 = final.tile([BL, NSTEP], f32)
        nc.scalar.activation(l1, c1, AF.Ln, bias=eps_sb)
        l2 = final.tile([BL, NSTEP], f32)
        nc.scalar.activation(l2, c2, AF.Ln, bias=eps_sb)
        m = final.tile([BL, NSTEP], f32)
        nc.vector.tensor_sub(m, l1, l2)
        m2 = final.tile([BL, NSTEP], f32)
        nc.vector.tensor_mul(m2, m, gt_sb[:, 1:T])
        s_t = final.tile([BL, NSTEP], f32)
        nc.vector.tensor_add(s_t, m2, l2)
        red = final.tile([BL, 1], f32)
        nc.vector.tensor_reduce(red, s_t, axis=AX.X, op=ALU.add)
        nred = final.tile([BL, 1], f32)
        nc.vector.tensor_scalar_mul(nred, red, -1.0)
        nc.sync.dma_start(out_d, nred)

    if reps == 1:
        compute()
    else:
        with tc.For_i(0, reps, 1):
            compute()


_BUILT = {}


def _build(reps=1):
    if reps in _BUILT:
        return _BUILT[reps]
    nc = bacc.Bacc("TRN2", target_bir_lowering=False, debug=False,
                   enable_asserts=False, num_devices=NCORES)
    aps = (
        nc.dram_tensor("x", [T * BL, I], bf16, kind="ExternalInput").ap(),
        nc.dram_tensor("gt_t", [BL, T], f32, kind="ExternalInput").ap(),
        nc.dram_tensor("w_ih_t", [I, 3 * H], bf16, kind="ExternalInput").ap(),
        nc.dram_tensor("w_hh_t", [H, 3 * H], bf16, kind="ExternalInput").ap(),
        nc.dram_tensor("b_ih_t", [H, 3], f32, kind="ExternalInput").ap(),
        nc.dram_tensor("b_hh_t", [H, 3], f32, kind="ExternalInput").ap(),
        nc.dram_tensor("w_dec_t", [H, 1], bf16, kind="ExternalInput").ap(),
        nc.dram_tensor("b_dec", [1, 1], f32, kind="ExternalInput").ap(),
        nc.dram_tensor("nll_part", [BL, 1], f32, kind="ExternalOutput").ap(),
    )
    with tile.TileContext(nc) as tc, ExitStack() as ctx:
        _body(ctx, tc, aps, reps=reps)
    nc.compile()
    _BUILT[reps] = nc
    return nc


def kernel(x, gt, W_ih, W_hh, b_ih, b_hh, W_dec, b_dec):
    global LAST_RESULTS
    x = np.asarray(x, dtype=np.float32)
    gt = np.asarray(gt, dtype=np.float32)
    W_ih = np.asarray(W_ih, dtype=np.float32)
    W_hh = np.asarray(W_hh, dtype=np.float32)
    b_ih = np.asarray(b_ih, dtype=np.float32)
    b_hh = np.asarray(b_hh, dtype=np.float32)
    W_dec = np.asarray(W_dec, dtype=np.float32)
    b_dec = np.asarray(b_dec, dtype=np.float32)

    nc = _build()

    bf = ml_dtypes.bfloat16
    shared = {
        "w_ih_t": np.ascontiguousarray(W_ih.T).astype(bf),
        "w_hh_t": np.ascontiguousarray(W_hh.T).astype(bf),
        "b_ih_t": np.ascontiguousarray(b_ih.reshape(3, H).T),
        "b_hh_t": np.ascontiguousarray(b_hh.reshape(3, H).T),
        "w_dec_t": np.ascontiguousarray(W_dec.reshape(1, H).T).astype(bf),
        "b_dec": np.ascontiguousarray(b_dec.reshape(1, 1)),
    }
    in_maps = []
    for c in range(NCORES):
        b0 = c * BL
        in_maps.append(dict(
            shared,
            x=np.ascontiguousarray(x[:, b0:b0 + BL, :]).reshape(
                T * BL, I).astype(bf),
            gt_t=np.ascontiguousarray(gt[:, b0:b0 + BL, 0].T),
        ))

    res = run_bass_kernel_spmd(nc, in_maps, core_ids=list(range(NCORES)))
    LAST_RESULTS = res
    total = sum(float(r["nll_part"].sum(dtype=np.float64)) for r in res.results)
    return np.float32(total / float(T * B))
